# revision 33
# baseline (speedup 1.0000x reference)
"""Trainium2 Bass kernel for the DWA middle layer (moe_routing).

Math (factored form of the reference; W_assembled is never materialized):
    t     = h_A @ V_flat^T                      # [B, N*R]
    s     = t * repeat(alpha, R, axis=1)        # [B, N*R]
    h_T   = s @ U_flat^T + h_A @ W_base^T + [alpha, 1] @ [bias_pool; b_base]
    out   = LayerNorm(h_A + gamma * h_T) * ln_scale + ln_bias

Sharding: data-parallel over the batch dim (32 rows per core, 8 cores);
weights replicated, streamed as fp8 e4m3 (scaled x32 on the host; the
scale is folded into alpha, the bias matrix and gamma).

Schedule (from perfetto/NTFF analysis):
  - Ring A (sync HWDGE) carries everything half-0 needs early
    (hdr, V.h0, fa, W.h0, bias rows); ring B (scalar) the rest, with
    W.h1 last in three pieces.  h_ps0 closes ~2.5us before h_ps1b, so
    half-0's LayerNorm legs run inside the stream.
  - Fused residual path: tiny DR identity matmuls accumulate
    (WSC/gamma)*(x8 + r8) into the h_ps tiles (r8 = fp8 residual of
    h_A, combined quantization ~2^-8), so the LayerNorm squares and
    final normalize read PSUM directly and no h_A f32 stream is
    needed.  Gated by _is_fused (WSC/gamma must be fp8-exact).
  - The LN mean is host-precomputed from h_A (the gamma*h_T mean
    contribution is ~3e-4 relative - negligible), removing the sum
    accumulators and three ops from the critical stats chain.
  - Warmup matmuls cover the PE HAM ramp until the first V chunk lands.
  - ACT tables (Square, Sqrt) are force-loaded at kernel start via
    high-priority warm activations on const inputs.
  - Toolchain constraints found the hard way: pool-engine (gpsimd)
    scalar_tensor_tensor and any DVE op reading the SAME PSUM tile as
    both tensor inputs crash the NEFF compiler; squares of PSUM data
    therefore go through ACT (one PSUM read) or a copied SBUF strip.
"""

import os
from contextlib import ExitStack

import ml_dtypes
import numpy as np

import concourse.bacc as bacc
import concourse.mybir as mybir
import concourse.tile as tile
from concourse import bass_utils, masks

F32 = mybir.dt.float32
F8 = mybir.dt.float8e4
NP_F8 = ml_dtypes.float8_e4m3

D = 1024          # d_A == d_B
B_CORE = 32       # batch rows per core
N_EXP = 64        # experts
R_RANK = 16       # rank per expert
N_CORES = 8
KT = 8            # 128-deep contraction tiles
JT = 4            # DoubleRow 256-deep contraction tiles
NH = 2            # output halves of 512
WSC = 16.0        # fp8 weight scale (folded into alpha/bias/gamma)
XW = 2 * B_CORE * JT  # 256 columns of h_A^T tiles
HH = JT * D       # 4096 cols per weight half
N_WU = int(os.environ.get("DWA_WARMUP_MM", "4"))  # PE warm-up matmuls
N_CD = int(os.environ.get("DWA_COOLDOWN_MM", "10"))  # PE cool-down matmuls

# ra columns (ring A piece 1): x8 | r8 | al | idd | V.h0
RA_X8 = 0
RA_R8 = XW                      # 256
RA_AL = 2 * XW                  # 512
RA_I3 = RA_AL + N_EXP           # 576
RA_V0 = RA_I3 + 512             # 1088
RA_COLS = RA_V0 + HH            # 5184
# wa columns (ring A piece 3): W.h0 | bb (bias rows + alpha^T)
WA_BB = HH
WA_COLS = HH + D + B_CORE       # 5152
# rb columns (ring B): V.h1 | U.h0 | U.h1
RB_V1 = 0
RB_U0 = HH
RB_U1 = 2 * HH
RB_COLS = 3 * HH
# W.h1 piece split: j01 rides ring B (last), j23 rides ring A
WB_J23 = 2 * D
# f columns: gmc | m | -m^2/D  (fused path; general path appends h_A)
FA_G = 0
FA_M = 1
FA_MV = 2
FA_COLS = 3
FG_COLS = 3 + D

_COMPILED = {}


def _build(general_ln, fused):
    nc = bacc.Bacc("TRN2", debug=False, num_devices=N_CORES,
                   enable_partition_id=False)

    ra_d = nc.dram_tensor("ra", [128, RA_COLS], F8, kind="ExternalInput")
    rb_d = nc.dram_tensor("rb", [128, RB_COLS], F8, kind="ExternalInput")
    wa_d = nc.dram_tensor("wa", [128, WA_COLS], F8, kind="ExternalInput")
    wb_d = nc.dram_tensor("wb", [128, HH], F8, kind="ExternalInput")
    fa_d = nc.dram_tensor("fa", [B_CORE, FA_COLS if fused else FG_COLS],
                          F32, kind="ExternalInput")
    if general_ln:
        lns_d = nc.dram_tensor("lns", [1, D], F32, kind="ExternalInput")
        lnb_d = nc.dram_tensor("lnb", [1, D], F32, kind="ExternalInput")
    else:
        lns_d = lnb_d = None
    out_d = nc.dram_tensor("out", [B_CORE, D], F32, kind="ExternalOutput")

    with ExitStack() as ctx:
        tc = ctx.enter_context(tile.TileContext(nc))
        _emit(ctx, tc, general_ln, fused, ra_d, rb_d, wa_d, wb_d, fa_d,
              lns_d, lnb_d, out_d)

    nc.compile()
    return nc


def _emit(ctx, tc, general_ln, fused, ra_d, rb_d, wa_d, wb_d, fa_d,
          lns_d, lnb_d, out_d):
    nc = tc.nc
    MULT = mybir.AluOpType.mult
    ADD = mybir.AluOpType.add
    SUB = mybir.AluOpType.subtract
    SQ = mybir.ActivationFunctionType.Square
    SQRT = mybir.ActivationFunctionType.Sqrt
    IDENT = mybir.ActivationFunctionType.Identity
    DR = mybir.MatmulPerfMode.DoubleRow

    wpool = ctx.enter_context(tc.tile_pool(name="weights", bufs=1))
    sm = ctx.enter_context(tc.tile_pool(name="small", bufs=1))
    pp = ctx.enter_context(tc.tile_pool(name="psum", bufs=1, space="PSUM"))

    ra_sb = wpool.tile([128, RA_COLS], F8, tag="ra")
    rb_sb = wpool.tile([128, RB_COLS], F8, tag="rb")
    wa_sb = wpool.tile([128, WA_COLS], F8, tag="wa")
    wb_sb = wpool.tile([128, HH], F8, tag="wb")
    x8_sb = ra_sb[:, :XW]
    al_sb = ra_sb[:, RA_AL:RA_V0]
    bp_sb = wa_sb[:N_EXP + 1, WA_BB:WA_BB + D]
    alt_sb = wa_sb[:N_EXP + 1, WA_BB + D:WA_BB + D + B_CORE]

    fa_sb = sm.tile([B_CORE, FA_COLS if fused else FG_COLS], F32, tag="fa")
    gmc = fa_sb[:, FA_G:FA_G + 1]
    m_c = fa_sb[:, FA_M:FA_M + 1]
    mvar_c = fa_sb[:, FA_MV:FA_MV + 1]
    idd_sb = ra_sb[:, RA_I3:RA_I3 + 512]
    z_sb = sm.tile([128, 2 * B_CORE], F8, tag="z8")
    g2_c = sm.tile([B_CORE, 1], F32, tag="g2c", name="g2c")

    wu_sb = sm.tile([128, 2 * B_CORE + 1024], F8, tag="wu")
    ident = sm.tile([B_CORE, B_CORE], F32, tag="ident")
    s_sb = sm.tile([B_CORE, D], F32, tag="s")
    st_sb = sm.tile([128, KT * B_CORE], F8, tag="st")
    hpre_sb = sm.tile([B_CORE, D], F32, tag="hpre")
    sq_sb = sm.tile([B_CORE, D], F32, tag="sq")
    out_sb = sm.tile([B_CORE, D], F32, tag="out")

    def c1(tag):
        return sm.tile([B_CORE, 1], F32, tag=tag, name=tag)

    sq0a, sq0b, sq1a, sq1b = c1("sq0a"), c1("sq0b"), c1("sq1a"), c1("sq1b")
    sq0t_c, pre01_c = c1("sq0t"), c1("pre01")
    ssqt_c, var_c = c1("ssqtc"), c1("varc")
    std_c, istd_c, nmi_c = c1("stdc"), c1("istdc"), c1("nmic")
    eps_c, warm_c, warm2_c = c1("epsc"), c1("warmc"), c1("warm2c")
    if general_ln:
        lnsr_sb = sm.tile([B_CORE, D], F32, tag="lnsr")
        lnbr_sb = sm.tile([B_CORE, D], F32, tag="lnbr")
        y_sb = sm.tile([B_CORE, D], F32, tag="y")
        t2_sb = sm.tile([B_CORE, D], F32, tag="t2")

    # ---- DMA issue: 4 big pieces per ring (each extra piece costs
    # ~0.7-1.4us of ring stall); no soft-DGE loads (they poison early
    # stream bandwidth).
    # Ring A (sync):   hdr+V.h0 | f (h_A) | W.h0+bb | W.h1 j23  (1.62 MB)
    # Ring B (scalar): V.h1 | U.h0 | U.h1 | W.h1 j01            (1.83 MB)
    nc.sync.dma_start(out=ra_sb[:], in_=ra_d.ap())
    nc.sync.dma_start(out=wa_sb[:], in_=wa_d.ap())
    nc.sync.dma_start(out=wb_sb[:, WB_J23:], in_=wb_d.ap()[:, WB_J23:])
    nc.sync.dma_start(out=wb_sb[:, :WB_J23], in_=wb_d.ap()[:, :WB_J23])

    nc.scalar.dma_start(out=fa_sb[:], in_=fa_d.ap())
    nc.scalar.dma_start(out=rb_sb[:, :RB_U0], in_=rb_d.ap()[:, :RB_U0])
    nc.scalar.dma_start(out=rb_sb[:, RB_U1:], in_=rb_d.ap()[:, RB_U1:])
    nc.scalar.dma_start(out=rb_sb[:, RB_U0:RB_U1],
                        in_=rb_d.ap()[:, RB_U0:RB_U1])
    if general_ln:
        nc.scalar.dma_start(out=lnsr_sb[:],
                            in_=lns_d.ap().broadcast_to([B_CORE, D]))
        nc.scalar.dma_start(out=lnbr_sb[:],
                            in_=lnb_d.ap().broadcast_to([B_CORE, D]))
    # ---- tiny setup; ACT tables (Square, Sqrt) force-loaded NOW ----
    with tc.high_priority():
        nc.gpsimd.memset(eps_c[:], 1e-5)
        nc.vector.memset(z_sb[:], 0.0)
        nc.scalar.activation(warm_c[:], eps_c[:], SQ)
        nc.scalar.activation(warm2_c[:], eps_c[:], SQRT, bias=eps_c[:],
                             scale=1.0)
        nc.vector.memset(wu_sb[:], 0.25)
    masks.make_identity(nc, ident[:])

    def dr_view(ap):
        return ap.rearrange("p (two n) -> p two n", two=2)

    def dr_lhs(x_sb, j):
        off = j * 2 * B_CORE
        return dr_view(x_sb[:, off:off + 2 * B_CORE])

    wu_ps = pp.tile([B_CORE, 512], F32, tag="wu", name="wu_ps")
    t_ps = [pp.tile([B_CORE, 512], F32, tag=f"t{h}", name=f"t_ps{h}")
            for h in range(NH)]
    h_ps0 = pp.tile([B_CORE, 512], F32, tag="h0", name="h_ps0")
    h_ps1 = pp.tile([B_CORE, 512], F32, tag="h1", name="h_ps1")
    tr_ps = [pp.tile([128, 128], F32, tag=f"tr{h}", name=f"tr_ps{h}")
             for h in range(NH)]

    # ---- PE warm-up: cover the HAM ramp until the first V chunk ----
    wu_lhs = dr_view(wu_sb[:, :2 * B_CORE])
    wu_rhs = dr_view(wu_sb[:, 2 * B_CORE:])
    for _ in range(N_WU):
        nc.tensor.matmul(wu_ps[:], wu_lhs, wu_rhs,
                         start=True, stop=True, perf_mode=DR)

    def v_chunk(h, j):
        if h == 0:
            return dr_view(ra_sb[:, RA_V0 + j * D:RA_V0 + (j + 1) * D])
        return dr_view(rb_sb[:, RB_V1 + j * D:RB_V1 + (j + 1) * D])

    def u_chunk(h, j):
        off = (RB_U0 if h == 0 else RB_U1) + j * D
        return dr_view(rb_sb[:, off:off + D])

    def w_chunk(h, j):
        w_sb = wa_sb if h == 0 else wb_sb
        return dr_view(w_sb[:, j * D:(j + 1) * D])

    # ---- t = h_A @ V^T (V.h0 rides ring A piece 1, lands first) ----
    for h in (0, 1):
        for j in range(JT):
            nc.tensor.matmul(t_ps[h][:], dr_lhs(x8_sb, j), v_chunk(h, j),
                             start=(j == 0), stop=(j == JT - 1),
                             perf_mode=DR)
    for h in (0, 1):
        # s = t * (alpha/4) * 0.125, transposed into st (fp8)
        o3 = s_sb[:, 512 * h:512 * (h + 1)].rearrange(
            "p (n r) -> p n r", r=R_RANK)
        i3 = t_ps[h][:].rearrange("p (n r) -> p n r", r=R_RANK)
        a3 = al_sb[:B_CORE, 32 * h:32 * (h + 1)].unsqueeze(-1).broadcast_to(
            [B_CORE, 32, R_RANK])
        nc.vector.scalar_tensor_tensor(
            out=o3, in0=i3, scalar=4.0 / WSC, in1=a3, op0=MULT, op1=MULT)
        for kk in range(4):
            k = 4 * h + kk
            nc.tensor.transpose(tr_ps[h][:, 32 * kk:32 * (kk + 1)],
                                s_sb[:, 128 * k:128 * (k + 1)], ident[:])
        nc.vector.tensor_copy(st_sb[:, 128 * h:128 * (h + 1)], tr_ps[h][:])

    # ---- residual fold (fused): h_ps[h] slices opened by tiny identity
    # matmuls 320*(x8+r8), so hpre = gmc*h_ps includes h_A directly ----
    if fused:
        # zero-open both h_ps tiles, then accumulate (WSC/gamma)*(x8+r8)
        # via DR matmuls against the dual-diagonal idd block
        for hp in (h_ps0, h_ps1):
            nc.tensor.matmul(hp[:], dr_view(z_sb[:]),
                             dr_view(wu_sb[:, 2 * B_CORE:]),
                             start=True, stop=False, perf_mode=DR)
        r8_sb = ra_sb[:, RA_R8:RA_AL]
        for h in range(NH):
            hp = h_ps0 if h == 0 else h_ps1
            for j in (2 * h, 2 * h + 1):
                sl = slice(256 * (j - 2 * h), 256 * (j - 2 * h) + 256)
                nc.tensor.matmul(hp[:, sl], dr_lhs(x8_sb, j),
                                 dr_view(idd_sb), start=False, stop=False,
                                 perf_mode=DR)
                nc.tensor.matmul(hp[:, sl], dr_lhs(r8_sb, j),
                                 dr_view(idd_sb), start=False, stop=False,
                                 perf_mode=DR)
        nc.vector.tensor_scalar(out=g2_c[:], in0=gmc, scalar1=gmc,
                                scalar2=1.0 / D, op0=MULT, op1=MULT)

    # ---- h_ps0 partial: W.h0 only (wa lands early ~12.4) ----
    for j in range(JT):
        nc.tensor.matmul(h_ps0[:], dr_lhs(x8_sb, j), w_chunk(0, j),
                         start=(not fused and j == 0), stop=False,
                         perf_mode=DR)

    # ---- h_ps1 group, arrival order: W.h1 j23 (ring A ~14.5) opens,
    # U.h1 (ring B 3rd, ~15.9), bias1, W.h1 j01 (~17.1) closes ----
    for j in (2, 3):
        nc.tensor.matmul(h_ps1[:], dr_lhs(x8_sb, j), w_chunk(1, j),
                         start=(not fused and j == 2), stop=False,
                         perf_mode=DR)
    for j in range(JT):
        nc.tensor.matmul(h_ps1[:], dr_lhs(st_sb, j), u_chunk(1, j),
                         start=False, stop=False, perf_mode=DR)
    nc.tensor.matmul(h_ps1[:], alt_sb[:], bp_sb[:, 512:],
                     start=False, stop=False)
    for j in (0, 1):
        nc.tensor.matmul(h_ps1[:], dr_lhs(x8_sb, j), w_chunk(1, j),
                         start=False, stop=(j == 1), perf_mode=DR)

    # ---- LayerNorm half 1 (early half: runs inside U.h0 closers) ----
    def ha(lo, hi):
        return fa_sb[:, 3 + lo:3 + hi]

    if fused:
        nc.scalar.activation(sq_sb[:, 512:768], h_ps1[:, 0:256], SQ,
                             accum_out=sq1a[:])
        nc.vector.tensor_copy(hpre_sb[:, 768:1024], h_ps1[:, 256:512])
        nc.vector.scalar_tensor_tensor(
            out=sq_sb[:, 768:1024], in0=hpre_sb[:, 768:1024], scalar=1.0,
            in1=hpre_sb[:, 768:1024], op0=MULT, op1=MULT, accum_out=sq1b[:])
    else:
        nc.vector.scalar_tensor_tensor(
            out=hpre_sb[:, 512:768], in0=h_ps1[:, 0:256], scalar=gmc,
            in1=ha(512, 768), op0=MULT, op1=ADD)
        nc.vector.scalar_tensor_tensor(
            out=hpre_sb[:, 768:1024], in0=h_ps1[:, 256:512], scalar=gmc,
            in1=ha(768, 1024), op0=MULT, op1=ADD)
        nc.scalar.activation(sq_sb[:, 512:768], hpre_sb[:, 512:768], SQ,
                             accum_out=sq1a[:])
        nc.vector.scalar_tensor_tensor(
            out=sq_sb[:, 768:1024], in0=hpre_sb[:, 768:1024], scalar=1.0,
            in1=hpre_sb[:, 768:1024], op0=MULT, op1=MULT, accum_out=sq1b[:])
    nc.vector.tensor_scalar(out=sq0t_c[:], in0=sq1a[:], scalar1=sq1b[:],
                            scalar2=None, op0=ADD)
    if not fused:
        nc.vector.tensor_scalar(out=pre01_c[:], in0=sq0t_c[:],
                                scalar1=mvar_c[:], scalar2=None, op0=ADD)

    # ---- h_ps0 closers: U.h0 (ring B last, ~17.7), bias0 ----
    for j in range(JT):
        nc.tensor.matmul(h_ps0[:], dr_lhs(st_sb, j), u_chunk(0, j),
                         start=False, stop=False, perf_mode=DR)
    nc.tensor.matmul(h_ps0[:], alt_sb[:], bp_sb[:, :512],
                     start=False, stop=True)

    # ---- LayerNorm half 0 (critical tail) ----
    if fused:
        nc.scalar.activation(sq_sb[:, 0:256], h_ps0[:, 0:256], SQ,
                             accum_out=sq0a[:])
        nc.vector.tensor_copy(hpre_sb[:, 256:512], h_ps0[:, 256:512])
        nc.vector.scalar_tensor_tensor(
            out=sq_sb[:, 256:512], in0=hpre_sb[:, 256:512], scalar=1.0,
            in1=hpre_sb[:, 256:512], op0=MULT, op1=MULT, accum_out=sq0b[:])
    else:
        nc.vector.scalar_tensor_tensor(
            out=hpre_sb[:, 0:256], in0=h_ps0[:, 0:256], scalar=gmc,
            in1=ha(0, 256), op0=MULT, op1=ADD)
        nc.scalar.activation(sq_sb[:, 0:256], hpre_sb[:, 0:256], SQ,
                             accum_out=sq0a[:])
        nc.vector.scalar_tensor_tensor(
            out=hpre_sb[:, 256:512], in0=h_ps0[:, 256:512], scalar=gmc,
            in1=ha(256, 512), op0=MULT, op1=ADD)
        nc.vector.scalar_tensor_tensor(
            out=sq_sb[:, 256:512], in0=hpre_sb[:, 256:512], scalar=1.0,
            in1=hpre_sb[:, 256:512], op0=MULT, op1=MULT, accum_out=sq0b[:])

    # stats: fused: std = sqrt((gmc^2/D)*ssq_raw + (eps-(m/D)^2));
    # general: std = sqrt(ssqt/D + eps) with -m^2/D pre-added via pre01
    if fused:
        nc.vector.tensor_scalar(out=ssqt_c[:], in0=sq0b[:], scalar1=sq0a[:],
                                scalar2=sq0t_c[:], op0=ADD, op1=ADD)
        nc.scalar.activation(std_c[:], ssqt_c[:], SQRT, bias=mvar_c[:],
                             scale=g2_c[:])
    else:
        nc.vector.tensor_scalar(out=ssqt_c[:], in0=sq0b[:], scalar1=sq0a[:],
                                scalar2=pre01_c[:], op0=ADD, op1=ADD)
        nc.scalar.activation(std_c[:], ssqt_c[:], SQRT, bias=eps_c[:],
                             scale=1.0 / D)
    nc.vector.reciprocal(istd_c[:], std_c[:])
    nc.vector.tensor_scalar(out=nmi_c[:], in0=m_c[:], scalar1=istd_c[:],
                            scalar2=-1.0 / D, op0=MULT, op1=MULT)

    # ---- final normalize ----
    if general_ln:
        for h in range(NH):
            sl = slice(512 * h, 512 * (h + 1))
            nc.vector.scalar_tensor_tensor(
                out=t2_sb[:, sl], in0=lnsr_sb[:, sl], scalar=nmi_c[:],
                in1=lnbr_sb[:, sl], op0=MULT, op1=ADD)
            nc.vector.scalar_tensor_tensor(
                out=y_sb[:, sl], in0=hpre_sb[:, sl], scalar=istd_c[:],
                in1=lnsr_sb[:, sl], op0=MULT, op1=MULT)
            nc.vector.tensor_add(out_sb[:, sl], y_sb[:, sl], t2_sb[:, sl])
        nc.sync.dma_start(out=out_d.ap()[:, :512], in_=out_sb[:, :512])
        nc.scalar.dma_start(out=out_d.ap()[:, 512:], in_=out_sb[:, 512:])
        return

    if fused:
        # out = (gmc*istd)*h_ps + nmi, straight from PSUM
        sc2 = var_c  # reuse a spare [32,1] tile
        nc.gpsimd.tensor_scalar(out=sc2[:], in0=istd_c[:], scalar1=gmc,
                                scalar2=None, op0=MULT)
        nc.scalar.activation(out_sb[:, 0:512], h_ps0[:], IDENT,
                             scale=sc2[:], bias=nmi_c[:])
        nc.vector.scalar_tensor_tensor(
            out=out_sb[:, 512:1024], in0=h_ps1[:], scalar=sc2[:],
            in1=nmi_c.broadcast_to([B_CORE, 512]), op0=MULT, op1=ADD)
        nc.scalar.dma_start(out=out_d.ap()[:, :512], in_=out_sb[:, :512])
        nc.sync.dma_start(out=out_d.ap()[:, 512:], in_=out_sb[:, 512:])
        _cooldown(nc, wu_ps, wu_lhs, wu_rhs, DR)
        return

    nc.scalar.activation(out_sb[:, 0:256], hpre_sb[:, 0:256], IDENT,
                         scale=istd_c[:], bias=nmi_c[:])
    nc.vector.tensor_scalar(out=out_sb[:, 256:1024], in0=hpre_sb[:, 256:1024],
                            scalar1=istd_c[:], scalar2=nmi_c[:],
                            op0=MULT, op1=ADD)
    nc.scalar.dma_start(out=out_d.ap()[:, :256], in_=out_sb[:, :256])
    nc.sync.dma_start(out=out_d.ap()[:, 256:], in_=out_sb[:, 256:])


def _cooldown(nc, wu_ps, wu_lhs, wu_rhs, DR):
    # Dummy matmuls during the otherwise-idle LN tail keep the PE HAM
    # activity window busy so the NRT postamble's Tensor-engine
    # semaphore resets run at the warm clock.
    for _ in range(N_CD):
        nc.tensor.matmul(wu_ps[:], wu_lhs, wu_rhs,
                         start=True, stop=True, perf_mode=DR)


def _dr_layout(m, scale):
    """[1024 k, 1024 out] f32 -> [128, (h j i n)] fp8 DoubleRow layout."""
    a = np.asarray(m * scale, dtype=NP_F8)
    # k -> (j, i, p), out -> (h, n); final [p, h, j, i, n]
    a = a.reshape(JT, 2, 128, NH, 512).transpose(2, 3, 0, 1, 4)
    return np.ascontiguousarray(a.reshape(128, KT * D))


def _prep_in_maps(inputs, general_ln, fused):
    def f32c(x):
        return np.ascontiguousarray(np.asarray(x, dtype=np.float32))

    h_a = f32c(inputs["h_A"])
    alpha = f32c(inputs["alpha"])
    pool = np.asarray(inputs["pool_vectors"], dtype=np.float32)
    w_base = np.asarray(inputs["W_base"], dtype=np.float32)

    # pool_vectors rows: [U_n (D*R) | V_n (R*D) | bias_n (D)]
    u = pool[:, :D * R_RANK].reshape(N_EXP, D, R_RANK)
    v = pool[:, D * R_RANK:2 * D * R_RANK].reshape(N_EXP, R_RANK, D)
    bias_pool = pool[:, 2 * D * R_RANK:]                    # [64, D]
    bb = np.asarray(inputs["b_base"], dtype=np.float32).reshape(1, D)
    # fp8 weights are scaled x32; alpha carries 1/32, so the bias rows
    # need x(32*32) for the pool part and x32 for b_base
    bp = np.concatenate([bias_pool * (4 * WSC), bb * (4 * WSC)], axis=0)
    vt = _dr_layout(v.reshape(N_EXP * R_RANK, D).T, WSC)   # [a, (n,r)]
    wt = _dr_layout(w_base.T, WSC)                          # [a, c]
    ut = _dr_layout(u.transpose(0, 2, 1).reshape(N_EXP * R_RANK, D), WSC)
    gm_raw = float(np.asarray(inputs["gamma"], dtype=np.float32))
    gm = gm_raw / WSC
    i8 = np.zeros((128, 512), np.float32)
    if fused:
        c = WSC / gm_raw
        for p in range(128):
            i8[p, p] = c
            i8[p, 384 + p] = c
    i8 = np.asarray(i8, dtype=NP_F8)

    in_maps = []
    for k in range(N_CORES):
        rows = slice(B_CORE * k, B_CORE * (k + 1))
        xt = h_a[rows].T                                    # [1024, 32]
        x8f = np.ascontiguousarray(
            xt.reshape(JT, 2, 128, B_CORE).transpose(2, 0, 1, 3)
            .reshape(128, XW))
        x8 = np.asarray(x8f, dtype=NP_F8)
        r8 = np.asarray(x8f - np.asarray(x8, np.float32), dtype=NP_F8)
        alt = np.concatenate(
            [alpha[rows] / 4.0, np.full((B_CORE, 1), 0.25, np.float32)],
            axis=1).T
        bbb = np.concatenate([bp, alt], axis=1)             # [65, 1056]
        bb8 = np.zeros((128, D + B_CORE), np.float32)
        bb8[:N_EXP + 1] = bbb
        al8 = np.zeros((128, N_EXP), np.float32)
        al8[:B_CORE] = alpha[rows] / 4.0
        ra = np.concatenate(
            [x8, r8, np.asarray(al8, dtype=NP_F8), i8, vt[:, :HH]], axis=1)
        rb = np.concatenate([vt[:, HH:], ut[:, :HH], ut[:, HH:]], axis=1)
        wa = np.concatenate([wt[:, :HH], np.asarray(bb8, dtype=NP_F8)],
                            axis=1)
        # LN mean is dominated by h_A (gamma*h_T mean contribution is
        # ~3e-4 relative): host-precompute m = sum(h_A) and msq = (m/D)^2*D
        msum = h_a[rows].sum(axis=1, keepdims=True)         # [32, 1]
        if fused:
            mvar = np.float32(1e-5) - (msum / D) ** 2
        else:
            mvar = -msum * msum / D
        scal = [np.full((B_CORE, 1), gm, np.float32), msum, mvar]
        fa = np.concatenate(
            scal if fused else scal + [h_a[rows]], axis=1)
        im = {
            "ra": np.ascontiguousarray(ra),
            "rb": np.ascontiguousarray(rb),
            "wa": np.ascontiguousarray(wa),
            "wb": np.ascontiguousarray(wt[:, HH:]),
            "fa": f32c(fa),
        }
        if general_ln:
            im["lns"] = f32c(inputs["ln_scale"]).reshape(1, D)
            im["lnb"] = f32c(inputs["ln_bias"]).reshape(1, D)
        in_maps.append(im)
    return in_maps


def _is_general_ln(inputs):
    lns = np.asarray(inputs["ln_scale"], dtype=np.float32)
    lnb = np.asarray(inputs["ln_bias"], dtype=np.float32)
    return not (np.all(lns == 1.0) and np.all(lnb == 0.0))


def _is_fused(inputs):
    gm = float(np.asarray(inputs["gamma"], dtype=np.float32))
    if gm == 0.0 or not np.isfinite(gm):
        return False
    c = np.float32(WSC / gm)
    return abs(c) <= 240 and float(np.asarray(c, dtype=NP_F8)) == float(c)


def get_compiled(general_ln=False, fused=False):
    key = (bool(general_ln), bool(fused))
    if key not in _COMPILED:
        _COMPILED[key] = _build(*key)
    return _COMPILED[key]


def kernel(**inputs):
    general_ln = _is_general_ln(inputs)
    fused = _is_fused(inputs) and not general_ln
    nc = get_compiled(general_ln, fused)
    in_maps = _prep_in_maps(inputs, general_ln, fused)
    res = bass_utils.run_bass_kernel_spmd(
        nc, in_maps, core_ids=list(range(N_CORES)))
    return np.concatenate([r["out"] for r in res.results], axis=0)
